# revision 14
# baseline (speedup 1.0000x reference)
"""Trainium2 Bass kernel for MllamaTextSelfAttention (B=1, S=2048, HID=4096,
32 Q heads / 8 KV heads, HD=128, RoPE, causal mask, GQA).

Sharding: tensor-parallel over heads across 8 NeuronCores. Core c computes
Q heads [4c, 4c+4) and KV head c, plus the matching slice of the output
projection; the 8 partial outputs are summed on the host.

v3 dataflow (per core, bf16 matmul operands, fp32 PSUM accumulation):
  - All inputs host-packed into SBUF-tile layout and converted to bf16 so
    every load is one large DMA (FWL-eligible stationaries), ordered so the
    first matmul's dependencies land first; wo loads during attention.
  - Phase 1 processes stripes in PAIRS with weight-stationary sharing: for
    each hidden k-tile the wk/wv/wq slice is loaded once and multiplied
    against both stripes' h tiles (the redundant second InstLdweights is
    removed by a post-scheduling dedupe pass). KV pass (4 PSUM banks) then
    Q pass (2 tags x 2 bufs); RoPE applied per (pair, tensor) right out of
    PSUM; V PE-transposed after the Q pass reusing the q PSUM banks.
  - Phase 2 (per stripe, per head-pair): S^T[k,q] = kT.T @ qT with the kT
    tile stationary shared across the pair; diagonal tiles compute only the
    live [128*dd, 512) column sub-range; exp on ACT straight from PSUM
    (bf16 out), causal zeroing via a precomputed 0/1 bf16 mask multiply on
    DVE; denominator via ones-column matmul packing two heads into one PSUM
    bank (partitions 0/32); O^T = V-tile.T @ E accumulated over k;
    normalization via rank-1 ones x recip matmul + DVE multiply.
  - Phase 3: output projection with stationary reuse: per (s-tile, head)
    the ot slice stays stationary across 4 moving wo slices, 2 groups of 4
    PSUM banks double-buffered; PSUM->bf16 copies alternate ACT/DVE; 16 row
    DMAs write the bf16 partial y.
"""

import math
import os
import sys

for _p in (
    "/opt/trn_rl_repo",
    "/root/.axon_site",
    "/root/.axon_site/_ro/trn_rl_repo",
    "/root/.axon_site/_ro/pypackages",
):
    if os.path.isdir(_p) and _p not in sys.path:
        sys.path.append(_p)

import numpy as np
import ml_dtypes
from contextlib import ExitStack

import concourse.bass as bass
import concourse.tile as tile
from concourse import mybir
from concourse.bass_utils import run_bass_kernel_spmd
from concourse.masks import make_identity

F32 = mybir.dt.float32
BF16 = mybir.dt.bfloat16
ACTF = mybir.ActivationFunctionType
ALU = mybir.AluOpType

B, S, HID = 1, 2048, 4096
NH, NKV, HD = 32, 8, 128
NCORES = 8
QH = NH // NCORES          # 4 q heads per core
SS = 512                   # sequence stripe
NQS = S // SS              # 4 stripes
NKT = S // 128             # 16 k tiles
KH = HID // 128            # 32 hidden-dim k tiles
HH_ = KH * SS // 2         # half-stripe free size (16 k-tiles)
NEG = -1e9


def _split_multi_waits(nc: bass.Bass):
    """Walrus in this container encodes at most ONE sync-wait command per
    instruction. Hoist extra waits onto injected same-engine NoOps placed
    immediately before the instruction; engines are in-order so the
    semantics are unchanged."""
    n = 0
    for fn in nc.m.functions:
        for bb in fn.blocks:
            out = []
            for inst in bb.instructions:
                si = inst.sync_info
                if si is not None and si.on_wait and len(si.on_wait) > 1:
                    waits = list(si.on_wait)
                    for w in waits[:-1]:
                        n += 1
                        nop = mybir.InstNoOp(name=f"I-swait-{n}", ins=[], outs=[])
                        nop.engine = inst.engine
                        nop.sync_info = mybir.SyncInfo(on_wait=[w], on_update=[])
                        out.append(nop)
                    si.on_wait = [waits[-1]]
                out.append(inst)
            bb.instructions[:] = out
    return nc


def _dedupe_ldweights(nc: bass.Bass):
    """The Tile legalizer emits one InstLdweights per matmul. Consecutive
    matmuls issued with the same stationary operand reload the PE array
    needlessly (~54-107ns each on HW). Drop the redundant loads: the PE
    array retains its weights across InstMatmult. Redundant loads carrying
    sync info become PE NoOps (sync position in the PE stream preserved);
    sync-free ones are deleted outright."""
    n = 0
    for fn in nc.m.functions:
        for bb in fn.blocks:
            out = []
            last_sig = None
            for inst in bb.instructions:
                if getattr(inst, "engine", None) == mybir.EngineType.PE:
                    nm = type(inst).__name__
                    if nm == "InstLdweights":
                        w = inst.ins[-1]
                        sig = (
                            str(w.memref),
                            w.offset,
                            str(w.ap),
                            str(w.dtype),
                            str(inst.perf_mode),
                            str(inst.is_transpose),
                            str(getattr(inst, "tile_position", None)),
                            str(getattr(inst, "tile_size", None)),
                        )
                        # a load into a sub-array tile leaves other tile
                        # positions' weights unknown to this tracker —
                        # only full-array loads are safe dedupe anchors
                        if sig[6] != "(0, 0)" or sig[7] != "(128, 128)":
                            last_sig = None
                            out.append(inst)
                            continue
                        if sig == last_sig:
                            si = inst.sync_info
                            if si is not None and (si.on_wait or si.on_update):
                                n += 1
                                nop = mybir.InstNoOp(
                                    name=f"I-dlw-{n}", ins=[], outs=[]
                                )
                                nop.engine = mybir.EngineType.PE
                                nop.sync_info = si
                                out.append(nop)
                            continue
                        last_sig = sig
                out.append(inst)
            bb.instructions[:] = out
    return nc


_BUILD_CACHE = {}
DEDUPE = True


def _build(causal: bool, split_waits: bool = True, loop_n=None) -> bass.Bass:
    key = (causal, split_waits, loop_n, DEDUPE)
    if key in _BUILD_CACHE:
        return _BUILD_CACHE[key]

    nc = bass.Bass()
    hS = [
        nc.dram_tensor(f"h{n}", [128, KH * SS], BF16, kind="ExternalInput")
        for n in range(NQS)
    ]
    wq = nc.dram_tensor("wq", [128, KH * SS], BF16, kind="ExternalInput")
    wk = nc.dram_tensor("wk", [128, KH * HD], BF16, kind="ExternalInput")
    wv = nc.dram_tensor("wv", [128, KH * HD], BF16, kind="ExternalInput")
    wo = nc.dram_tensor("wo", [128, QH * HID], BF16, kind="ExternalInput")
    trig = nc.dram_tensor("trig", [128, 2 * S], BF16, kind="ExternalInput")
    if not causal:
        maskT = nc.dram_tensor("maskT", [S, S], F32, kind="ExternalInput")
    y = nc.dram_tensor("y", [S, HID], BF16, kind="ExternalOutput")

    with tile.TileContext(nc) as tc, ExitStack() as ctx:
        if loop_n is not None:
            ctx.enter_context(tc.For_i(0, loop_n, 1))

        outer = ctx.enter_context(tc.tile_pool(name="outer", bufs=1))
        wq_sb = outer.tile([128, KH * SS], BF16)
        wk_sb = outer.tile([128, KH * HD], BF16)
        wv_sb = outer.tile([128, KH * HD], BF16)
        trig_sb = outer.tile([128, 2 * S], BF16)
        qT = outer.tile([128, QH * S], BF16)     # [d, h*s] rope'd
        kT = outer.tile([128, S], BF16)          # [d, s] rope'd
        v_sb = outer.tile([128, S], BF16)        # [s-within-tile, t*d]
        ot = outer.tile([128, QH * S], BF16)     # [d, h*s] normalized O^T
        ones_col = outer.tile([128, 1], BF16)
        ones_row = outer.tile([1, 128], BF16)
        id_bf = outer.tile([128, 128], BF16)
        d01 = outer.tile([128, QH * SS], BF16)   # causal 0/1 mask per dd

        # upfront bulk loads in consumption order (SP-triggered)
        nc.sync.dma_start(wk_sb[:], wk[:, :])
        nc.sync.dma_start(wv_sb[:], wv[:, :])

        nc.vector.memset(ones_col[:], 1.0)
        nc.vector.memset(ones_row[:], 1.0)
        make_identity(nc, id_bf[:])
        nc.gpsimd.memset(d01[:], 1.0)
        for dd in range(QH):
            # keep 1.0 where q - k >= 0: iota = j - p - 128*dd
            nc.gpsimd.affine_select(
                out=d01[:, dd * SS : (dd + 1) * SS],
                in_=d01[:, dd * SS : (dd + 1) * SS],
                pattern=[[1, SS]],
                compare_op=ALU.is_ge,
                fill=0.0,
                base=-(128 * dd),
                channel_multiplier=-1,
            )

        # ------- phase 1: QKV projections (stripe pairs) + RoPE + V^T -----
        with (
            tc.tile_pool(name="hstream", bufs=6) as hp,
            tc.tile_pool(name="stage", bufs=2) as sp_,
            tc.tile_pool(name="ps1", bufs=1, space="PSUM") as pp1,
            tc.tile_pool(name="psq", bufs=2, space="PSUM") as ppq,
        ):
            def rope(src_psum, dst):
                stg = sp_.tile([128, SS], BF16, tag="stg")
                nc.scalar.copy(stg[:], src_psum)
                t1 = sp_.tile([128, SS], BF16, tag="t1")
                t2 = sp_.tile([128, SS], BF16, tag="t2")
                nc.vector.tensor_copy(t1[0:64, :], stg[64:128, :])
                nc.vector.tensor_copy(t1[64:128, :], stg[0:64, :])
                nc.vector.tensor_mul(t1[:], t1[:], rope.ms)
                nc.vector.tensor_mul(t2[:], stg[:], rope.cs)
                nc.vector.tensor_add(dst, t1[:], t2[:])

            # h half-stripe tiles, DMA'd in KV-pass consumption order
            half = {}
            def load_half(n, j):
                t_ = hp.tile([128, HH_], BF16, name=f"hh{n}{j}", tag="hh")
                nc.sync.dma_start(t_[:], hS[n][:, j * HH_ : (j + 1) * HH_])
                half[(n, j)] = t_

            load_half(0, 0)
            load_half(1, 0)
            load_half(0, 1)
            load_half(1, 1)
            nc.sync.dma_start(trig_sb[:], trig[:, :])
            nc.sync.dma_start(wq_sb[:], wq[:, :])

            def hsl(n, k):
                t_ = half[(n, k // 16)]
                o = (k % 16) * SS
                return t_[:, o : o + SS]

            for pr in range(NQS // 2):
                a, b = 2 * pr, 2 * pr + 1
                # ---- KV pass: wk/wv stationary shared across the pair ----
                psk = [pp1.tile([128, SS], F32, name=f"psk{i}", tag=f"psk{i}") for i in range(2)]
                psv = [pp1.tile([128, SS], F32, name=f"psv{i}", tag=f"psv{i}") for i in range(2)]
                for k in range(KH):
                    st_, sp2 = (k == 0), (k == KH - 1)
                    wksl = wk_sb[:, k * HD : (k + 1) * HD]
                    wvsl = wv_sb[:, k * HD : (k + 1) * HD]
                    nc.tensor.matmul(psk[0][:], wksl, hsl(a, k), start=st_, stop=sp2)
                    nc.tensor.matmul(psk[1][:], wksl, hsl(b, k), start=st_, stop=sp2)
                    nc.tensor.matmul(psv[0][:], wvsl, hsl(a, k), start=st_, stop=sp2)
                    nc.tensor.matmul(psv[1][:], wvsl, hsl(b, k), start=st_, stop=sp2)

                vbf = []
                for i, n in enumerate((a, b)):
                    rope.cs = trig_sb[:, n * SS : (n + 1) * SS]
                    rope.ms = trig_sb[:, S + n * SS : S + (n + 1) * SS]
                    rope(psk[i][:], kT[:, n * SS : (n + 1) * SS])
                    vb = sp_.tile([128, SS], BF16, name=f"vb{i}", tag=f"vb{i}")
                    nc.scalar.copy(vb[:], psv[i][:])
                    vbf.append(vb)

                # ---- Q pass: wq stationary shared across the pair ----
                for m in range(QH):
                    psqa = ppq.tile([128, SS], F32, tag="qa")
                    psqb = ppq.tile([128, SS], F32, tag="qb")
                    for k in range(KH):
                        st_, sp2 = (k == 0), (k == KH - 1)
                        wqsl = wq_sb[:, k * SS + m * 128 : k * SS + (m + 1) * 128]
                        nc.tensor.matmul(psqa[:], wqsl, hsl(a, k), start=st_, stop=sp2)
                        nc.tensor.matmul(psqb[:], wqsl, hsl(b, k), start=st_, stop=sp2)
                    for n, ps in ((a, psqa), (b, psqb)):
                        rope.cs = trig_sb[:, n * SS : (n + 1) * SS]
                        rope.ms = trig_sb[:, S + n * SS : S + (n + 1) * SS]
                        rope(ps[:], qT[:, m * S + n * SS : m * S + (n + 1) * SS])
                    if pr == 0 and m < 2:
                        # prefetch next pair's first halves during Q pass
                        load_half(2 + m, 0)
                if pr == 0:
                    load_half(2, 1)
                    load_half(3, 1)

                # ---- V transposes (reuse q psum banks) ----
                for i in range(2):
                    n = (a, b)[i]
                    for j in range(SS // 128):
                        t4 = 4 * n + j
                        pst = ppq.tile([128, 128], BF16, tag=("qa", "qb")[i])
                        nc.tensor.transpose(
                            pst[:], vbf[i][:, j * 128 : (j + 1) * 128], id_bf[:]
                        )
                        nc.vector.tensor_copy(
                            v_sb[:, t4 * 128 : (t4 + 1) * 128], pst[:]
                        )

        # ---------------- phase 2: attention ----------------
        with tc.tile_pool(name="wop", bufs=1) as wop:
          wo_sb = wop.tile([128, QH * HID], BF16)
          nc.sync.dma_start(wo_sb[:], wo[:, :])
          with (
            tc.tile_pool(name="epool", bufs=2) as ep,
            tc.tile_pool(name="att", bufs=2) as ap_,
            tc.tile_pool(name="mrowp", bufs=1) as mp,
            tc.tile_pool(name="ps2s", bufs=2, space="PSUM") as pp2s,
            tc.tile_pool(name="ps2a", bufs=1, space="PSUM") as pp2a,
          ):
            for qs in range(NQS):
                nkt = 4 * qs + 4 if causal else NKT
                if not causal:
                    mrow = mp.tile([128, NKT * SS], F32, tag="mrow")
                    for t in range(NKT):
                        nc.sync.dma_start(
                            mrow[:, t * SS : (t + 1) * SS],
                            maskT[t * 128 : (t + 1) * 128, qs * SS : (qs + 1) * SS],
                        )
                for hp_ in range(QH // 2):
                    h0, h1 = 2 * hp_, 2 * hp_ + 1
                    e0 = ep.tile([128, NKT * SS], BF16, tag="e0")
                    e1 = ep.tile([128, NKT * SS], BF16, tag="e1")
                    for t in range(nkt):
                        ksl = kT[:, t * 128 : (t + 1) * 128]
                        diag = causal and t >= 4 * qs
                        off = 128 * (t - 4 * qs) if diag else 0
                        w_ = SS - off
                        for hh, hid_ in ((0, h0), (1, h1)):
                            qsl = qT[:, hid_ * S + qs * SS + off : hid_ * S + (qs + 1) * SS]
                            pss = pp2s.tile([128, SS], F32, tag=f"pss{hh}")
                            ee = (e0, e1)[hh]
                            nc.tensor.matmul(
                                pss[:, 0:w_], ksl, qsl, start=True, stop=True
                            )
                            dst = ee[:, t * SS + off : (t + 1) * SS]
                            if causal:
                                nc.scalar.activation(dst, pss[:, 0:w_], ACTF.Exp)
                                if diag:
                                    dd = t - 4 * qs
                                    if off:
                                        nc.vector.memset(
                                            ee[:, t * SS : t * SS + off], 0.0
                                        )
                                    nc.vector.tensor_mul(
                                        dst, dst,
                                        d01[:, dd * SS + off : (dd + 1) * SS],
                                    )
                            else:
                                nc.vector.tensor_add(
                                    pss[:], pss[:], mrow[:, t * SS : (t + 1) * SS]
                                )
                                nc.scalar.activation(dst, pss[:], ACTF.Exp)

                    psd = pp2a.tile([64, SS], F32, tag="psd")
                    pso0 = pp2a.tile([128, SS], F32, tag="pso0")
                    pso1 = pp2a.tile([128, SS], F32, tag="pso1")
                    # denominator: pre-sum groups of 4 E tiles on DVE (bf16
                    # SBUF 4x mode), one ones-matmul per group -> 4x fewer
                    # PE rows for the row-sum
                    ng = nkt // 4
                    for g in range(ng):
                        for (ee, prow) in ((e0, 0), (e1, 32)):
                            base = 4 * g * SS
                            s01 = ap_.tile([128, SS], BF16, tag="ds0")
                            s23 = ap_.tile([128, SS], BF16, tag="ds1")
                            sal = ap_.tile([128, SS], BF16, tag="ds2")
                            nc.vector.tensor_add(
                                s01[:], ee[:, base : base + SS],
                                ee[:, base + SS : base + 2 * SS],
                            )
                            nc.vector.tensor_add(
                                s23[:], ee[:, base + 2 * SS : base + 3 * SS],
                                ee[:, base + 3 * SS : base + 4 * SS],
                            )
                            nc.vector.tensor_add(sal[:], s01[:], s23[:])
                            nc.tensor.matmul(
                                psd[prow : prow + 1, :], ones_col[:], sal[:],
                                start=(g == 0), stop=(g == ng - 1),
                                skip_group_check=True,
                            )
                    for t in range(nkt):
                        st_, sp2 = (t == 0), (t == nkt - 1)
                        vsl = v_sb[:, t * 128 : (t + 1) * 128]
                        e0t = e0[:, t * SS : (t + 1) * SS]
                        e1t = e1[:, t * SS : (t + 1) * SS]
                        nc.tensor.matmul(pso0[:], vsl, e0t, start=st_, stop=sp2)
                        nc.tensor.matmul(pso1[:], vsl, e1t, start=st_, stop=sp2)

                    with nc.allow_low_precision(reason="bf16 recip feeds matmul"):
                        rec0 = ap_.tile([1, SS], BF16, tag="rec0")
                        rec1 = ap_.tile([1, SS], BF16, tag="rec1")
                        nc.vector.reciprocal(rec0[:], psd[0:1, :])
                        nc.vector.reciprocal(rec1[:], psd[32:33, :])
                    psb0 = pp2s.tile([128, SS], F32, tag="pss0")
                    psb1 = pp2s.tile([128, SS], F32, tag="pss1")
                    nc.tensor.matmul(
                        psb0[:], ones_row[:], rec0[:], start=True, stop=True
                    )
                    nc.tensor.matmul(
                        psb1[:], ones_row[:], rec1[:], start=True, stop=True
                    )
                    for (hh, pso, psb) in ((h0, pso0, psb0), (h1, pso1, psb1)):
                        od = ot[:, hh * S + qs * SS : hh * S + (qs + 1) * SS]
                        nc.vector.tensor_copy(od, pso[:])
                        nc.vector.tensor_mul(od, od, psb[:])

          # ------------- phase 3: output projection -------------
          with (
                tc.tile_pool(name="yout", bufs=2) as yp,
                tc.tile_pool(name="ps3", bufs=1, space="PSUM") as pp3,
          ):
                cp_engines = (
                    lambda o, i: nc.scalar.copy(o, i),
                    lambda o, i: nc.vector.tensor_copy(o, i),
                )
                rr = 0
                for st in range(NKT):
                    yt = yp.tile([128, HID], BF16, tag="yt")
                    for g in range(2):
                        psy = [
                            pp3.tile([128, SS], F32, name=f"psy{g}{j}", tag=f"psy{g}{j}")
                            for j in range(4)
                        ]
                        for hh in range(QH):
                            osl = ot[:, hh * S + st * 128 : hh * S + (st + 1) * 128]
                            for j in range(4):
                                nn = 4 * g + j
                                nc.tensor.matmul(
                                    psy[j][:],
                                    osl,
                                    wo_sb[:, hh * HID + nn * SS : hh * HID + (nn + 1) * SS],
                                    start=(hh == 0),
                                    stop=(hh == QH - 1),
                                )
                        for j in range(4):
                            nn = 4 * g + j
                            cp_engines[rr % 2](
                                yt[:, nn * SS : (nn + 1) * SS], psy[j][:]
                            )
                            rr += 1
                    nc.sync.dma_start(y[st * 128 : (st + 1) * 128, :], yt[:])

    if DEDUPE:
        _dedupe_ldweights(nc)
    if split_waits:
        _split_multi_waits(nc)
    _BUILD_CACHE[key] = nc
    return nc


def _causal_mask_ref() -> np.ndarray:
    return np.triu(np.full((S, S), NEG, np.float32), k=1)


def _pack(a: np.ndarray) -> np.ndarray:
    """[R, W] with R = 128*r -> [128, r*W] SBUF tile layout, bf16."""
    r = a.shape[0] // 128
    w = a.shape[1]
    out = a.reshape(r, 128, w).transpose(1, 0, 2).reshape(128, r * w)
    return np.ascontiguousarray(out.astype(ml_dtypes.bfloat16))


def make_in_maps(hidden_states, attention_mask, cos, sin, wq, wk, wv, wo):
    """Host-side sharding/packing. Returns (causal, in_maps)."""
    h = np.asarray(hidden_states, dtype=np.float32)[0]
    m2 = np.ascontiguousarray(np.asarray(attention_mask, dtype=np.float32)[0, 0])
    wq = np.asarray(wq, dtype=np.float32)
    wk = np.asarray(wk, dtype=np.float32)
    wv = np.asarray(wv, dtype=np.float32)
    wo = np.asarray(wo, dtype=np.float32)

    causal = bool(np.array_equal(m2, _causal_mask_ref()))
    hT = h.T  # [HID, S]
    cosT = np.asarray(cos, dtype=np.float32)[0].T  # [HD, S]
    sinT = np.asarray(sin, dtype=np.float32)[0].T
    msinT = np.concatenate([-sinT[0:64], sinT[64:128]], axis=0)
    trig = np.ascontiguousarray(
        np.concatenate([cosT, msinT], axis=1).astype(ml_dtypes.bfloat16)
    )
    sc = np.float32(1.0 / math.sqrt(HD))

    h_stripes = [
        _pack(np.ascontiguousarray(hT[:, n * SS : (n + 1) * SS])) for n in range(NQS)
    ]
    if not causal:
        mT = np.ascontiguousarray(m2.T)

    in_maps = []
    for c in range(NCORES):
        im = {
            "trig": trig,
            "wq": _pack(np.ascontiguousarray((wq[c * QH * HD : (c + 1) * QH * HD] * sc).T)),
            "wk": _pack(np.ascontiguousarray(wk[c * HD : (c + 1) * HD].T)),
            "wv": _pack(np.ascontiguousarray(wv[c * HD : (c + 1) * HD].T)),
            "wo": _pack(np.ascontiguousarray(wo[:, c * QH * HD : (c + 1) * QH * HD].T)),
        }
        for n in range(NQS):
            im[f"h{n}"] = h_stripes[n]
        if not causal:
            im["maskT"] = mT
        in_maps.append(im)
    return causal, in_maps


def kernel(hidden_states, attention_mask, cos, sin, wq, wk, wv, wo):
    causal, in_maps = make_in_maps(
        hidden_states, attention_mask, cos, sin, wq, wk, wv, wo
    )
    nc = _build(causal)
    res = run_bass_kernel_spmd(nc, in_maps, list(range(NCORES)))
    out = np.zeros((S, HID), np.float64)
    for c in range(NCORES):
        out += res.results[c]["y"].astype(np.float64)
    return out.reshape(B, S, HID).astype(np.float32)


# revision 18
# speedup vs baseline: 1.1285x; 1.1285x over previous
"""Trainium2 Bass kernel for MllamaTextSelfAttention (B=1, S=2048, HID=4096,
32 Q heads / 8 KV heads, HD=128, RoPE, causal mask, GQA).

Sharding: tensor-parallel over heads across 8 NeuronCores. Core c computes
Q heads [4c, 4c+4) and KV head c, plus the matching slice of the output
projection; the 8 partial outputs are summed on the host.

v3 dataflow (per core, bf16 matmul operands, fp32 PSUM accumulation):
  - All inputs host-packed into SBUF-tile layout and converted to bf16 so
    every load is one large DMA (FWL-eligible stationaries), ordered so the
    first matmul's dependencies land first; wo loads during attention.
  - Phase 1 processes stripes in PAIRS with weight-stationary sharing: for
    each hidden k-tile the wk/wv/wq slice is loaded once and multiplied
    against both stripes' h tiles (the redundant second InstLdweights is
    removed by a post-scheduling dedupe pass). KV pass (4 PSUM banks) then
    Q pass (2 tags x 2 bufs); RoPE applied per (pair, tensor) right out of
    PSUM; V PE-transposed after the Q pass reusing the q PSUM banks.
  - Phase 2 (per stripe, per head-pair): S^T[k,q] = kT.T @ qT with the kT
    tile stationary shared across the pair; diagonal tiles compute only the
    live [128*dd, 512) column sub-range; exp on ACT straight from PSUM
    (bf16 out), causal zeroing via a precomputed 0/1 bf16 mask multiply on
    DVE; denominator via ones-column matmul packing two heads into one PSUM
    bank (partitions 0/32); O^T = V-tile.T @ E accumulated over k;
    normalization via rank-1 ones x recip matmul + DVE multiply.
  - Phase 3: output projection with stationary reuse: per (s-tile, head)
    the ot slice stays stationary across 4 moving wo slices, 2 groups of 4
    PSUM banks double-buffered; PSUM->bf16 copies alternate ACT/DVE; 16 row
    DMAs write the bf16 partial y.
"""

import math
import os
import sys

for _p in (
    "/opt/trn_rl_repo",
    "/root/.axon_site",
    "/root/.axon_site/_ro/trn_rl_repo",
    "/root/.axon_site/_ro/pypackages",
):
    if os.path.isdir(_p) and _p not in sys.path:
        sys.path.append(_p)

import numpy as np
import ml_dtypes
from contextlib import ExitStack

import concourse.bass as bass
import concourse.tile as tile
from concourse import mybir
from concourse.bass_utils import run_bass_kernel_spmd
from concourse.masks import make_identity

F32 = mybir.dt.float32
BF16 = mybir.dt.bfloat16
ACTF = mybir.ActivationFunctionType
ALU = mybir.AluOpType

B, S, HID = 1, 2048, 4096
NH, NKV, HD = 32, 8, 128
NCORES = 8
QH = NH // NCORES          # 4 q heads per core
SS = 512                   # sequence stripe
NQS = S // SS              # 4 stripes
NKT = S // 128             # 16 k tiles
KH = HID // 128            # 32 hidden-dim k tiles
HH_ = KH * SS // 2         # half-stripe free size (16 k-tiles)
NEG = -1e9


def _split_multi_waits(nc: bass.Bass):
    """Walrus in this container encodes at most ONE sync-wait command per
    instruction. Hoist extra waits onto injected same-engine NoOps placed
    immediately before the instruction; engines are in-order so the
    semantics are unchanged."""
    n = 0
    for fn in nc.m.functions:
        for bb in fn.blocks:
            out = []
            for inst in bb.instructions:
                si = inst.sync_info
                if si is not None and si.on_wait and len(si.on_wait) > 1:
                    waits = list(si.on_wait)
                    for w in waits[:-1]:
                        n += 1
                        nop = mybir.InstNoOp(name=f"I-swait-{n}", ins=[], outs=[])
                        nop.engine = inst.engine
                        nop.sync_info = mybir.SyncInfo(on_wait=[w], on_update=[])
                        out.append(nop)
                    si.on_wait = [waits[-1]]
                out.append(inst)
            bb.instructions[:] = out
    return nc


def _dedupe_ldweights(nc: bass.Bass):
    """The Tile legalizer emits one InstLdweights per matmul. Consecutive
    matmuls issued with the same stationary operand reload the PE array
    needlessly (~54-107ns each on HW). Drop the redundant loads: the PE
    array retains its weights across InstMatmult. Redundant loads carrying
    sync info become PE NoOps (sync position in the PE stream preserved);
    sync-free ones are deleted outright."""
    n = 0
    for fn in nc.m.functions:
        for bb in fn.blocks:
            out = []
            last_sig = None
            for inst in bb.instructions:
                if getattr(inst, "engine", None) == mybir.EngineType.PE:
                    nm = type(inst).__name__
                    if nm == "InstLdweights":
                        w = inst.ins[-1]
                        sig = (
                            str(w.memref),
                            w.offset,
                            str(w.ap),
                            str(w.dtype),
                            str(inst.perf_mode),
                            str(inst.is_transpose),
                            str(getattr(inst, "tile_position", None)),
                            str(getattr(inst, "tile_size", None)),
                        )
                        # a load into a sub-array tile leaves other tile
                        # positions' weights unknown to this tracker —
                        # only full-array loads are safe dedupe anchors
                        if sig[6] != "(0, 0)" or sig[7] != "(128, 128)":
                            last_sig = None
                            out.append(inst)
                            continue
                        if sig == last_sig:
                            si = inst.sync_info
                            if si is not None and (si.on_wait or si.on_update):
                                n += 1
                                nop = mybir.InstNoOp(
                                    name=f"I-dlw-{n}", ins=[], outs=[]
                                )
                                nop.engine = mybir.EngineType.PE
                                nop.sync_info = si
                                out.append(nop)
                            continue
                        last_sig = sig
                out.append(inst)
            bb.instructions[:] = out
    return nc


_BUILD_CACHE = {}
DEDUPE = True


def _build(causal: bool, split_waits: bool = True, loop_n=None) -> bass.Bass:
    key = (causal, split_waits, loop_n, DEDUPE)
    if key in _BUILD_CACHE:
        return _BUILD_CACHE[key]

    nc = bass.Bass()
    hS = [
        nc.dram_tensor(f"h{n}", [128, KH * SS], BF16, kind="ExternalInput")
        for n in range(NQS)
    ]
    wq = nc.dram_tensor("wq", [128, KH * SS], BF16, kind="ExternalInput")
    wk = nc.dram_tensor("wk", [128, KH * HD], BF16, kind="ExternalInput")
    wv = nc.dram_tensor("wv", [128, KH * HD], BF16, kind="ExternalInput")
    wo = nc.dram_tensor("wo", [128, QH * HID], BF16, kind="ExternalInput")
    trig = nc.dram_tensor("trig", [128, 2 * S], BF16, kind="ExternalInput")
    if not causal:
        maskT = nc.dram_tensor("maskT", [S, S], F32, kind="ExternalInput")
    y = nc.dram_tensor("y", [S, HID], BF16, kind="ExternalOutput")

    with tile.TileContext(nc) as tc, ExitStack() as ctx:
        if loop_n is not None:
            ctx.enter_context(tc.For_i(0, loop_n, 1))

        outer = ctx.enter_context(tc.tile_pool(name="outer", bufs=1))
        wq_sb = outer.tile([128, KH * SS], BF16)
        wk_sb = outer.tile([128, KH * HD], BF16)
        wv_sb = outer.tile([128, KH * HD], BF16)
        trig_sb = outer.tile([128, 2 * S], BF16)
        qT = outer.tile([128, QH * S], BF16)     # [d, h*s] rope'd
        kT = outer.tile([128, S], BF16)          # [d, s] rope'd
        v_sb = outer.tile([128, S], BF16)        # [s-within-tile, t*d]
        ot = outer.tile([128, QH * S], BF16)     # [d, h*s] normalized O^T
        ones_col = outer.tile([128, 1], BF16)
        ones_row = outer.tile([1, 128], BF16)
        id_bf = outer.tile([128, 128], BF16)
        d01 = outer.tile([128, QH * SS], BF16)   # causal 0/1 mask per dd

        # upfront bulk loads in consumption order (SP-triggered)
        nc.sync.dma_start(wk_sb[:], wk[:, :])
        nc.sync.dma_start(wv_sb[:], wv[:, :])

        nc.vector.memset(ones_col[:], 1.0)
        nc.vector.memset(ones_row[:], 1.0)
        make_identity(nc, id_bf[:])
        nc.gpsimd.memset(d01[:], 1.0)
        for dd in range(QH):
            # keep 1.0 where q - k >= 0: iota = j - p - 128*dd
            nc.gpsimd.affine_select(
                out=d01[:, dd * SS : (dd + 1) * SS],
                in_=d01[:, dd * SS : (dd + 1) * SS],
                pattern=[[1, SS]],
                compare_op=ALU.is_ge,
                fill=0.0,
                base=-(128 * dd),
                channel_multiplier=-1,
            )

        # ------- phase 1: QKV projections (stripe pairs) + RoPE + V^T -----
        with (
            tc.tile_pool(name="hstream", bufs=6) as hp,
            tc.tile_pool(name="stage", bufs=2) as sp_,
            tc.tile_pool(name="ps1", bufs=1, space="PSUM") as pp1,
            tc.tile_pool(name="psq", bufs=2, space="PSUM") as ppq,
        ):
            def rope(src_psum, dst):
                stg = sp_.tile([128, SS], BF16, tag="stg")
                nc.scalar.copy(stg[:], src_psum)
                t1 = sp_.tile([128, SS], BF16, tag="t1")
                t2 = sp_.tile([128, SS], BF16, tag="t2")
                nc.vector.tensor_copy(t1[0:64, :], stg[64:128, :])
                nc.vector.tensor_copy(t1[64:128, :], stg[0:64, :])
                nc.vector.tensor_mul(t1[:], t1[:], rope.ms)
                nc.vector.tensor_mul(t2[:], stg[:], rope.cs)
                nc.vector.tensor_add(dst, t1[:], t2[:])

            # h half-stripe tiles, DMA'd in KV-pass consumption order
            half = {}
            def load_half(n, j):
                t_ = hp.tile([128, HH_], BF16, name=f"hh{n}{j}", tag="hh")
                nc.sync.dma_start(t_[:], hS[n][:, j * HH_ : (j + 1) * HH_])
                half[(n, j)] = t_

            load_half(0, 0)
            load_half(1, 0)
            load_half(0, 1)
            load_half(1, 1)
            nc.sync.dma_start(trig_sb[:], trig[:, :])
            nc.sync.dma_start(wq_sb[:], wq[:, :])

            def hsl(n, k):
                t_ = half[(n, k // 16)]
                o = (k % 16) * SS
                return t_[:, o : o + SS]

            for pr in range(NQS // 2):
                a, b = 2 * pr, 2 * pr + 1
                # ---- KV pass: wk/wv stationary shared across the pair ----
                psk = [pp1.tile([128, SS], F32, name=f"psk{i}", tag=f"psk{i}") for i in range(2)]
                psv = [pp1.tile([128, SS], F32, name=f"psv{i}", tag=f"psv{i}") for i in range(2)]
                for k in range(KH):
                    st_, sp2 = (k == 0), (k == KH - 1)
                    wksl = wk_sb[:, k * HD : (k + 1) * HD]
                    wvsl = wv_sb[:, k * HD : (k + 1) * HD]
                    nc.tensor.matmul(psk[0][:], wksl, hsl(a, k), start=st_, stop=sp2)
                    nc.tensor.matmul(psk[1][:], wksl, hsl(b, k), start=st_, stop=sp2)
                    nc.tensor.matmul(psv[0][:], wvsl, hsl(a, k), start=st_, stop=sp2)
                    nc.tensor.matmul(psv[1][:], wvsl, hsl(b, k), start=st_, stop=sp2)

                vbf = []
                for i, n in enumerate((a, b)):
                    rope.cs = trig_sb[:, n * SS : (n + 1) * SS]
                    rope.ms = trig_sb[:, S + n * SS : S + (n + 1) * SS]
                    rope(psk[i][:], kT[:, n * SS : (n + 1) * SS])
                    vb = sp_.tile([128, SS], BF16, name=f"vb{i}", tag=f"vb{i}")
                    nc.scalar.copy(vb[:], psv[i][:])
                    vbf.append(vb)

                # ---- Q pass: wq stationary shared across the pair ----
                for m in range(QH):
                    psqa = ppq.tile([128, SS], F32, tag="qa")
                    psqb = ppq.tile([128, SS], F32, tag="qb")
                    for k in range(KH):
                        st_, sp2 = (k == 0), (k == KH - 1)
                        wqsl = wq_sb[:, k * SS + m * 128 : k * SS + (m + 1) * 128]
                        nc.tensor.matmul(psqa[:], wqsl, hsl(a, k), start=st_, stop=sp2)
                        nc.tensor.matmul(psqb[:], wqsl, hsl(b, k), start=st_, stop=sp2)
                    for n, ps in ((a, psqa), (b, psqb)):
                        rope.cs = trig_sb[:, n * SS : (n + 1) * SS]
                        rope.ms = trig_sb[:, S + n * SS : S + (n + 1) * SS]
                        rope(ps[:], qT[:, m * S + n * SS : m * S + (n + 1) * SS])
                    if pr == 0 and m < 2:
                        # prefetch next pair's first halves during Q pass
                        load_half(2 + m, 0)
                if pr == 0:
                    load_half(2, 1)
                    load_half(3, 1)

                # ---- V transposes (reuse q psum banks) ----
                for i in range(2):
                    n = (a, b)[i]
                    for j in range(SS // 128):
                        t4 = 4 * n + j
                        pst = ppq.tile([128, 128], BF16, tag=("qa", "qb")[i])
                        nc.tensor.transpose(
                            pst[:], vbf[i][:, j * 128 : (j + 1) * 128], id_bf[:]
                        )
                        nc.vector.tensor_copy(
                            v_sb[:, t4 * 128 : (t4 + 1) * 128], pst[:]
                        )

        # ---------------- phase 2: attention ----------------
        with tc.tile_pool(name="wop", bufs=1) as wop:
          wo_sb = wop.tile([128, QH * HID], BF16)
          nc.sync.dma_start(wo_sb[:], wo[:, :])
          with (
            tc.tile_pool(name="epool", bufs=2) as ep,
            tc.tile_pool(name="att", bufs=2) as ap_,
            tc.tile_pool(name="dsum", bufs=3) as dsp,
            tc.tile_pool(name="mrowp", bufs=1) as mp,
            tc.tile_pool(name="ps2s", bufs=2, space="PSUM") as pp2s,
            tc.tile_pool(name="ps2a", bufs=1, space="PSUM") as pp2a,
          ):
            for qs in range(NQS):
                nkt = 4 * qs + 4 if causal else NKT
                if not causal:
                    mrow = mp.tile([128, NKT * SS], F32, tag="mrow")
                    for t in range(NKT):
                        nc.sync.dma_start(
                            mrow[:, t * SS : (t + 1) * SS],
                            maskT[t * 128 : (t + 1) * 128, qs * SS : (qs + 1) * SS],
                        )
                for hp_ in range(QH // 2):
                    h0, h1 = 2 * hp_, 2 * hp_ + 1
                    e0 = ep.tile([128, NKT * SS], BF16, tag="e0")
                    e1 = ep.tile([128, NKT * SS], BF16, tag="e1")
                    # denominator: 2-wide E-tile pre-sums on DVE issued inside
                    # the scores loop; the ones-matmul for group g lands two
                    # iterations later so its DVE dep is long resolved
                    psd = pp2a.tile([64, SS], F32, tag="psd")
                    ng = nkt // 2
                    sals = ([], [])

                    def issue_psd(g):
                        for hh2, prow in ((0, 0), (1, 32)):
                            nc.tensor.matmul(
                                psd[prow : prow + 1, :], ones_col[:],
                                sals[hh2][g][:],
                                start=(g == 0), stop=(g == ng - 1),
                                skip_group_check=True,
                            )

                    for t in range(nkt):
                        ksl = kT[:, t * 128 : (t + 1) * 128]
                        diag = causal and t >= 4 * qs
                        off = 128 * (t - 4 * qs) if diag else 0
                        w_ = SS - off
                        for hh, hid_ in ((0, h0), (1, h1)):
                            qsl = qT[:, hid_ * S + qs * SS + off : hid_ * S + (qs + 1) * SS]
                            pss = pp2s.tile([128, SS], F32, tag=f"pss{hh}")
                            ee = (e0, e1)[hh]
                            nc.tensor.matmul(
                                pss[:, 0:w_], ksl, qsl, start=True, stop=True
                            )
                            dst = ee[:, t * SS + off : (t + 1) * SS]
                            if causal:
                                nc.scalar.activation(dst, pss[:, 0:w_], ACTF.Exp)
                                if diag:
                                    dd = t - 4 * qs
                                    if off:
                                        nc.vector.memset(
                                            ee[:, t * SS : t * SS + off], 0.0
                                        )
                                    nc.vector.tensor_mul(
                                        dst, dst,
                                        d01[:, dd * SS + off : (dd + 1) * SS],
                                    )
                            else:
                                nc.vector.tensor_add(
                                    pss[:], pss[:], mrow[:, t * SS : (t + 1) * SS]
                                )
                                nc.scalar.activation(dst, pss[:], ACTF.Exp)
                        if t % 2 == 1:
                            g = t // 2
                            for hh2 in range(2):
                                ee2 = (e0, e1)[hh2]
                                sal = dsp.tile(
                                    [128, SS], BF16, name=f"ds{hh2}", tag=f"ds{hh2}"
                                )
                                nc.vector.tensor_add(
                                    sal[:],
                                    ee2[:, (t - 1) * SS : t * SS],
                                    ee2[:, t * SS : (t + 1) * SS],
                                )
                                sals[hh2].append(sal)
                            if g >= 1:
                                issue_psd(g - 1)
                    issue_psd(ng - 1)

                    pso0 = pp2a.tile([128, SS], F32, tag="pso0")
                    pso1 = pp2a.tile([128, SS], F32, tag="pso1")
                    for t in range(nkt):
                        st_, sp2 = (t == 0), (t == nkt - 1)
                        vsl = v_sb[:, t * 128 : (t + 1) * 128]
                        e0t = e0[:, t * SS : (t + 1) * SS]
                        e1t = e1[:, t * SS : (t + 1) * SS]
                        nc.tensor.matmul(pso0[:], vsl, e0t, start=st_, stop=sp2)
                        nc.tensor.matmul(pso1[:], vsl, e1t, start=st_, stop=sp2)

                    with nc.allow_low_precision(reason="bf16 recip feeds matmul"):
                        rec0 = ap_.tile([1, SS], BF16, tag="rec0")
                        rec1 = ap_.tile([1, SS], BF16, tag="rec1")
                        nc.vector.reciprocal(rec0[:], psd[0:1, :])
                        nc.vector.reciprocal(rec1[:], psd[32:33, :])
                    psb0 = pp2s.tile([128, SS], F32, tag="pss0")
                    psb1 = pp2s.tile([128, SS], F32, tag="pss1")
                    nc.tensor.matmul(
                        psb0[:], ones_row[:], rec0[:], start=True, stop=True
                    )
                    nc.tensor.matmul(
                        psb1[:], ones_row[:], rec1[:], start=True, stop=True
                    )
                    for (hh, pso, psb) in ((h0, pso0, psb0), (h1, pso1, psb1)):
                        od = ot[:, hh * S + qs * SS : hh * S + (qs + 1) * SS]
                        nc.vector.tensor_copy(od, pso[:])
                        nc.vector.tensor_mul(od, od, psb[:])

          # ------------- phase 3: output projection -------------
          with (
                tc.tile_pool(name="yout", bufs=2) as yp,
                tc.tile_pool(name="ps3", bufs=1, space="PSUM") as pp3,
          ):
                cp_engines = (
                    lambda o, i: nc.scalar.copy(o, i),
                    lambda o, i: nc.vector.tensor_copy(o, i),
                )
                rr = 0
                for st in range(NKT):
                    yt = yp.tile([128, HID], BF16, tag="yt")
                    for g in range(2):
                        psy = [
                            pp3.tile([128, SS], F32, name=f"psy{g}{j}", tag=f"psy{g}{j}")
                            for j in range(4)
                        ]
                        for hh in range(QH):
                            osl = ot[:, hh * S + st * 128 : hh * S + (st + 1) * 128]
                            for j in range(4):
                                nn = 4 * g + j
                                nc.tensor.matmul(
                                    psy[j][:],
                                    osl,
                                    wo_sb[:, hh * HID + nn * SS : hh * HID + (nn + 1) * SS],
                                    start=(hh == 0),
                                    stop=(hh == QH - 1),
                                )
                        for j in range(4):
                            nn = 4 * g + j
                            cp_engines[rr % 2](
                                yt[:, nn * SS : (nn + 1) * SS], psy[j][:]
                            )
                            rr += 1
                    nc.sync.dma_start(y[st * 128 : (st + 1) * 128, :], yt[:])

    if DEDUPE:
        _dedupe_ldweights(nc)
    if split_waits:
        _split_multi_waits(nc)
    _BUILD_CACHE[key] = nc
    return nc


def _causal_mask_ref() -> np.ndarray:
    return np.triu(np.full((S, S), NEG, np.float32), k=1)


def _pack(a: np.ndarray) -> np.ndarray:
    """[R, W] with R = 128*r -> [128, r*W] SBUF tile layout, bf16."""
    r = a.shape[0] // 128
    w = a.shape[1]
    out = a.reshape(r, 128, w).transpose(1, 0, 2).reshape(128, r * w)
    return np.ascontiguousarray(out.astype(ml_dtypes.bfloat16))


def make_in_maps(hidden_states, attention_mask, cos, sin, wq, wk, wv, wo):
    """Host-side sharding/packing. Returns (causal, in_maps)."""
    h = np.asarray(hidden_states, dtype=np.float32)[0]
    m2 = np.ascontiguousarray(np.asarray(attention_mask, dtype=np.float32)[0, 0])
    wq = np.asarray(wq, dtype=np.float32)
    wk = np.asarray(wk, dtype=np.float32)
    wv = np.asarray(wv, dtype=np.float32)
    wo = np.asarray(wo, dtype=np.float32)

    causal = bool(np.array_equal(m2, _causal_mask_ref()))
    hT = h.T  # [HID, S]
    cosT = np.asarray(cos, dtype=np.float32)[0].T  # [HD, S]
    sinT = np.asarray(sin, dtype=np.float32)[0].T
    msinT = np.concatenate([-sinT[0:64], sinT[64:128]], axis=0)
    trig = np.ascontiguousarray(
        np.concatenate([cosT, msinT], axis=1).astype(ml_dtypes.bfloat16)
    )
    sc = np.float32(1.0 / math.sqrt(HD))

    h_stripes = [
        _pack(np.ascontiguousarray(hT[:, n * SS : (n + 1) * SS])) for n in range(NQS)
    ]
    if not causal:
        mT = np.ascontiguousarray(m2.T)

    in_maps = []
    for c in range(NCORES):
        im = {
            "trig": trig,
            "wq": _pack(np.ascontiguousarray((wq[c * QH * HD : (c + 1) * QH * HD] * sc).T)),
            "wk": _pack(np.ascontiguousarray(wk[c * HD : (c + 1) * HD].T)),
            "wv": _pack(np.ascontiguousarray(wv[c * HD : (c + 1) * HD].T)),
            "wo": _pack(np.ascontiguousarray(wo[:, c * QH * HD : (c + 1) * QH * HD].T)),
        }
        for n in range(NQS):
            im[f"h{n}"] = h_stripes[n]
        if not causal:
            im["maskT"] = mT
        in_maps.append(im)
    return causal, in_maps


def kernel(hidden_states, attention_mask, cos, sin, wq, wk, wv, wo):
    causal, in_maps = make_in_maps(
        hidden_states, attention_mask, cos, sin, wq, wk, wv, wo
    )
    nc = _build(causal)
    res = run_bass_kernel_spmd(nc, in_maps, list(range(NCORES)))
    out = np.zeros((S, HID), np.float64)
    for c in range(NCORES):
        out += res.results[c]["y"].astype(np.float64)
    return out.reshape(B, S, HID).astype(np.float32)
